# revision 11
# baseline (speedup 1.0000x reference)
"""Bass/Trainium2 kernel for nn_DecorrelationGradient.

Reference computation (KAPPA = 0.5):
    out = (1-k)*(gram - diag_ms) + k*(diag_ms - 1)
        = 0.5 * (X^T X / N) - 0.5          (diag terms cancel algebraically)

with X = x.reshape(N, d), N = 8*2048 = 16384, d = 768.

Strategy (data-parallel over the sample axis, 8 cores):
  - core c gets x[c] : [2048, 768] f32
  - computes the upper-triangle blocks of its partial Gram P_c = x_c^T x_c
    on the PE (float32r matmuls = single-pass fp32, PSUM accumulation over
    16 k-tiles; k-outer/i-inner so each x tile is consumed right after its
    DMA lands)
  - fused scale+bias on the PSUM->SBUF copy:  t = P_c * (0.5/N) - 0.5/8
  - packed triangle [128, 2688] f32 -> DRAM, ReduceScatter(add) over 8 cores
  - each core outputs its 16-partition-row slice of the reduced packed
    triangle; the host concatenates the slices and unpacks the symmetric
    matrix (pure indexing, no arithmetic).
"""

import numpy as np

import concourse.bacc as bacc
import concourse.bass as bass  # noqa: F401
import concourse.tile as tile
from concourse import mybir
from concourse.bass_utils import run_bass_kernel_spmd

P = 128
D = 768
NSHARD = 2048          # samples per core
KT = NSHARD // P       # 16 k-tiles
NB = D // P            # 6 row/col blocks
NCORES = 8
NTOT = 8 * 2048
SCALE = 0.5 / NTOT     # 2**-15, exact
BIAS = -0.5 / NCORES   # -0.0625, exact; RS adds 8 copies -> -0.5

# packed upper-triangle blocks (i, j) with j >= i, row-major in i
TRI_BLOCKS = [(i, j) for i in range(NB) for j in range(i, NB)]
NTRI = len(TRI_BLOCKS)          # 21
TRI_W = NTRI * P                # 2688 packed columns
ROWS_PER_CORE = P // NCORES     # 16 partition rows of the packed triangle


def _split_free(width):
    """Split a moving free-dim into chunks <= 512, each >= 256
    (float32r runs 1 cycle/row only at free size >= 256)."""
    if width <= 512:
        return [(0, width)]
    if width == 640:
        return [(0, 384), (384, 640)]
    if width == 768:
        return [(0, 512), (512, 768)]
    raise ValueError(width)


def _build():
    nc = bacc.Bacc(num_devices=NCORES)

    x_sh = nc.dram_tensor(
        "x_shard", [NSHARD, D], mybir.dt.float32, kind="ExternalInput"
    )
    out_sh = nc.dram_tensor(
        "out_shard", [ROWS_PER_CORE, TRI_W], mybir.dt.float32, kind="ExternalOutput"
    )

    f32 = mybir.dt.float32
    f32r = mybir.dt.float32r

    with tile.TileContext(nc) as tc:
        with (
            tc.tile_pool(name="xp", bufs=KT) as xpool,
            tc.tile_pool(name="ps", bufs=1, space="PSUM") as pspool,
            tc.tile_pool(name="acc", bufs=1) as accpool,
            tc.tile_pool(name="dram", bufs=1, space="DRAM") as dpool,
        ):
            # load the full shard into SBUF as 16 [128, 768] tiles.
            # float32r = same bits as f32 (no cast; the dtype tag selects the
            # PE's single-pass fp32 matmul mode), so HWDGE full-rate DMA.
            xt = []
            for k in range(KT):
                xtile = xpool.tile([P, D], f32r, tag="x", name=f"x{k}")
                nc.sync.dma_start(
                    out=xtile[:], in_=x_sh[k * P : (k + 1) * P, :].bitcast(f32r)
                )
                xt.append(xtile)

            tri = accpool.tile([P, TRI_W], f32)  # packed scaled triangle
            g_in = dpool.tile([P, TRI_W], f32, name="g_in")
            g_out = dpool.tile([ROWS_PER_CORE, TRI_W], f32, name="g_out")

            # psum accumulators, one per row-block; exactly 8 PSUM banks.
            # row-block i covers G[i-block, j-blocks j>=i] = cols 128*i..768;
            # i == NB-1 widens to 256 cols (recomputes block (5,4)) to keep
            # the f32r moving free-dim >= 256; only (5,5) is copied out.
            pss, col0 = [], []
            for i in range(NB):
                c0 = P * i if i < NB - 1 else D - 2 * P
                pss.append(pspool.tile([P, D - c0], f32, tag=f"ps{i}", name=f"ps{i}"))
                col0.append(c0)

            # k-outer / i-inner: each x tile is fully consumed right after
            # its DMA arrives, so PE overlaps the load stream
            for k in range(KT):
                for i in range(NB):
                    lhsT = xt[k][:, P * i : P * (i + 1)]
                    for s0, s1 in _split_free(D - col0[i]):
                        nc.tensor.matmul(
                            pss[i][:, s0:s1],
                            lhsT=lhsT,
                            rhs=xt[k][:, col0[i] + s0 : col0[i] + s1],
                            start=(k == 0),
                            stop=(k == KT - 1),
                        )

            # fused (x * SCALE + BIAS) on the PSUM->SBUF copy, then stream
            # each packed slice to DRAM immediately
            off = 0
            for i in range(NB):
                wout = (NB - i) * P
                W = D - col0[i]
                src = pss[i][:, W - wout : W]
                nc.scalar.activation(
                    out=tri[:, off : off + wout],
                    in_=src,
                    func=mybir.ActivationFunctionType.Copy,
                    scale=SCALE,
                    bias=BIAS,
                )
                nc.sync.dma_start(
                    out=g_in[:, off : off + wout], in_=tri[:, off : off + wout]
                )
                off += wout
            assert off == TRI_W

            nc.gpsimd.collective_compute(
                "ReduceScatter",
                mybir.AluOpType.add,
                replica_groups=[list(range(NCORES))],
                ins=[g_in.opt()],
                outs=[g_out.opt()],
            )
            nc.sync.dma_start(out=out_sh[:, :], in_=g_out[:])

    nc.finalize()  # Bacc: run reg-alloc + wait-legalization passes
    return nc


_NC_CACHE = None

# test-harness hooks (harness calls kernel() only; these stay defaults there)
RUN_KWARGS = {}
LAST_RESULTS = None


def _get_nc():
    global _NC_CACHE
    if _NC_CACHE is None:
        _NC_CACHE = _build()
    return _NC_CACHE


def kernel(x: np.ndarray) -> np.ndarray:
    global LAST_RESULTS
    x = np.ascontiguousarray(np.asarray(x, dtype=np.float32))
    assert x.shape == (NCORES, NSHARD, D)

    nc = _get_nc()
    in_maps = [{"x_shard": x[c]} for c in range(NCORES)]
    res = run_bass_kernel_spmd(
        nc, in_maps, core_ids=list(range(NCORES)), **RUN_KWARGS
    )
    LAST_RESULTS = res

    # gather: concatenate the per-core partition-row slices of the packed
    # triangle, then unpack the symmetric matrix (indexing only)
    packed = np.concatenate(
        [res.results[c]["out_shard"] for c in range(NCORES)], axis=0
    )  # [128, 2688]
    packed = packed.reshape(P, NTRI, P).transpose(1, 0, 2)  # [21, 128, 128]

    out = np.empty((D, D), dtype=np.float32)
    for b, (i, j) in enumerate(TRI_BLOCKS):
        blk = packed[b]
        out[P * i : P * (i + 1), P * j : P * (j + 1)] = blk
        if j != i:
            out[P * j : P * (j + 1), P * i : P * (i + 1)] = blk.T
    return out
